# revision 6
# baseline (speedup 1.0000x reference)
"""Trainium2 Bass kernel for the PostProcess problem.

Computation (per batch element):
  pred = argmax(pred_logits[b], axis=-1)          # [L] over V=51267
  token state machine -> one-hot return_logits [100, 256]
  scores = 1 - softmax(return_logits)[..., -1]
  boxes  = cxcywh_to_xyxy(pred_boxes) * [w,h,w,h]
  labels = ones

Sharding: pure data parallel, 2 batch elements per core on 8 cores.

Argmax strategy (memory-bound regime): stream the logits through SBUF once,
reduce to per-128-element-subchunk maxima on DVE (1 full scan), find the
winning subchunk with tiny max/max_index ops, then re-fetch just the winning
512B subchunk per row via indirect DMA and max_index it.  DVE work ~1.03
scans instead of 2 (max+max_index over everything), so the kernel stays
DMA-bound at ~105MB/core.
"""

import sys

if "/opt/trn_rl_repo" not in sys.path:
    sys.path.insert(0, "/opt/trn_rl_repo")

from contextlib import ExitStack

import numpy as np

import concourse.bacc as bacc
import concourse.bass as bass
import concourse.tile as tile
from concourse import mybir
from concourse.bass_utils import run_bass_kernel_spmd

V = 51267
L = 256
NB = 2              # batch elements per core
RW = NB * L         # 512 token rows per core
NCORES = 8
TEXT = 50265.0      # text vocab size; open=50265, close=50266, bbox>50266
NQ = 100
NP = 256
SUB = 128           # subchunk size for hierarchical argmax
SFULL = 400         # full subchunks per row (400*128 = 51200)
TAIL = V - SFULL * SUB   # 67
STOT = SFULL + 1    # 401 subchunk maxima per row
PADF = 128          # flat padding so the tail gather never reads OOB
NFLAT = RW * V + PADF
CH_OFF = [0, 12800, 25600, 38400]
CH_COLS = [12800, 12800, 12800, 12867]

f32 = mybir.dt.float32
i32 = mybir.dt.int32
u32 = mybir.dt.uint32
OP = mybir.AluOpType
AX = mybir.AxisListType
ACTF = mybir.ActivationFunctionType

_CACHE = {}


def _build():
    nc = bacc.Bacc("TRN2", target_bir_lowering=False, debug=False)
    lg = nc.dram_tensor("lg", [NFLAT], f32, kind="ExternalInput")
    bx = nc.dram_tensor("bx", [NB, NQ, 4], f32, kind="ExternalInput")
    sc = nc.dram_tensor("sc", [NB, 4], f32, kind="ExternalInput")
    scores = nc.dram_tensor("scores", [NB, NQ], f32, kind="ExternalOutput")
    oboxes = nc.dram_tensor("oboxes", [NB, NQ, 4], f32, kind="ExternalOutput")

    lgap = lg.ap()                                      # [NFLAT]
    lgm = lgap[0 : RW * V].rearrange("(r v) -> r v", v=V)   # [512, V]
    lgcol = lgap.unsqueeze(1)                           # [NFLAT, 1]

    with tile.TileContext(nc) as tc, ExitStack() as ctx:
        cpool = ctx.enter_context(tc.tile_pool(name="consts", bufs=1))
        spool = ctx.enter_context(tc.tile_pool(name="stream", bufs=3))
        mpool = ctx.enter_context(tc.tile_pool(name="smax", bufs=2))
        wpool = ctx.enter_context(tc.tile_pool(name="work", bufs=2))
        ppool = ctx.enter_context(tc.tile_pool(name="persist", bufs=1))
        qpool = ctx.enter_context(tc.tile_pool(name="psmall", bufs=1, space="PSUM"))
        hpool = ctx.enter_context(tc.tile_pool(name="phot", bufs=2, space="PSUM"))

        # ---------------- constants ----------------
        it_a = cpool.tile([128, 1], i32)
        it_b = cpool.tile([128, NQ], i32)
        it_c = cpool.tile([128, NP], i32)
        it_d = cpool.tile([128, 128], i32)
        it_e = cpool.tile([128, 1], i32)
        iota_pv = cpool.tile([128, 1], f32)     # p * V
        nc.gpsimd.iota(it_a[:], [[1, 1]], channel_multiplier=V)
        nc.vector.tensor_copy(iota_pv[:], it_a[:])
        iota4 = cpool.tile([128, NQ], f32)      # 0,4,...,396 per partition
        nc.gpsimd.iota(it_b[:], [[4, NQ]], channel_multiplier=0)
        nc.vector.tensor_copy(iota4[:], it_b[:])
        iota256 = cpool.tile([128, NP], f32)    # 0..255 per partition
        nc.gpsimd.iota(it_c[:], [[1, NP]], channel_multiplier=0)
        nc.vector.tensor_copy(iota256[:], it_c[:])
        irow = cpool.tile([128, 128], f32)      # 0..127 per partition
        nc.gpsimd.iota(it_d[:], [[1, 128]], channel_multiplier=0)
        nc.vector.tensor_copy(irow[:], it_d[:])
        pcol = cpool.tile([128, 1], f32)        # partition index
        nc.gpsimd.iota(it_e[:], [[1, 1]], channel_multiplier=1)
        nc.vector.tensor_copy(pcol[:], it_e[:])
        # LT[k, m] = 1 if k <= m  (inclusive prefix-sum matrix)
        LT = cpool.tile([128, 128], f32)
        nc.vector.tensor_scalar(LT[:], irow[:], pcol[:, 0:1], None, OP.is_ge)
        ONES = cpool.tile([128, 128], f32)
        nc.vector.memset(ONES[:], 1.0)

        # ---------------- phase 1+2: streaming subchunk max + argmax ----------------
        pred_all = ppool.tile([128, 4], f32)    # col r: token rows r*128..r*128+127
        for r in range(4):
            smax = mpool.tile([128, STOT], f32, tag="smax")
            for c in range(4):
                cols = CH_COLS[c]
                X = spool.tile([128, 12867], f32, tag="X")
                nc.sync.dma_start(
                    out=X[:, 0:cols],
                    in_=lgm[r * 128 : (r + 1) * 128, CH_OFF[c] : CH_OFF[c] + cols],
                )
                nc.vector.tensor_reduce(
                    out=smax[:, c * 100 : (c + 1) * 100],
                    in_=X[:, 0:12800].rearrange("p (s k) -> p s k", k=SUB),
                    axis=AX.X,
                    op=OP.max,
                )
                if c == 3:
                    nc.vector.tensor_reduce(
                        out=smax[:, SFULL : SFULL + 1],
                        in_=X[:, 12800:12867],
                        axis=AX.X,
                        op=OP.max,
                    )
            top8 = wpool.tile([128, 8], f32, tag="top8")
            kidx8 = wpool.tile([128, 8], u32, tag="kidx8")
            nc.vector.max(out=top8[:], in_=smax[:, 0:STOT])
            nc.vector.max_index(out=kidx8[:], in_max=top8[:], in_values=smax[:, 0:STOT])
            kf = wpool.tile([128, 1], f32, tag="kf")
            nc.vector.tensor_copy(kf[:], kidx8[:, 0:1].bitcast(i32))
            offf = wpool.tile([128, 1], f32, tag="offf")
            nc.vector.scalar_tensor_tensor(
                out=offf[:], in0=kf[:], scalar=float(SUB), in1=iota_pv[:],
                op0=OP.mult, op1=OP.add,
            )
            offi = wpool.tile([128, 1], i32, tag="offi")
            nc.vector.tensor_copy(offi[:], offf[:])
            gath = wpool.tile([128, SUB], f32, tag="gath")
            nc.gpsimd.indirect_dma_start(
                out=gath[:],
                out_offset=None,
                in_=lgcol,
                in_offset=bass.IndirectOffsetOnAxis(ap=offi[:, 0:1], axis=0),
                element_offset=r * 128 * V,
            )
            widx8 = wpool.tile([128, 8], u32, tag="widx8")
            nc.vector.max_index(out=widx8[:], in_max=top8[:], in_values=gath[:])
            wf = wpool.tile([128, 1], f32, tag="wf")
            nc.vector.tensor_copy(wf[:], widx8[:, 0:1].bitcast(i32))
            nc.vector.scalar_tensor_tensor(
                out=pred_all[:, r : r + 1], in0=kf[:], scalar=float(SUB), in1=wf[:],
                op0=OP.mult, op1=OP.add,
            )

        # ---------------- phase 3: token state machine ----------------
        # group g = r: batch b = g//2, half h = g%2 (token l = h*128 + p)
        iso = ppool.tile([128, 4], f32)
        isc = ppool.tile([128, 4], f32)
        isb = ppool.tile([128, 4], f32)
        ist = ppool.tile([128, 4], f32)
        nc.vector.tensor_scalar(iso[:], pred_all[:], TEXT, None, OP.is_equal)
        nc.vector.tensor_scalar(isc[:], pred_all[:], TEXT + 1.0, None, OP.is_equal)
        nc.vector.tensor_scalar(isb[:], pred_all[:], TEXT + 1.0, None, OP.is_gt)
        nc.vector.tensor_scalar(ist[:], pred_all[:], TEXT, None, OP.is_lt)
        # signals tile: cols [0:4]=delta(g), [4:8]=is_bbox(g), [8:12]=is_text(g)
        sg = ppool.tile([128, 12], f32)
        nc.vector.tensor_sub(sg[:, 0:4], iso[:], isc[:])
        nc.vector.tensor_copy(sg[:, 4:8], isb[:])
        nc.vector.tensor_copy(sg[:, 8:12], ist[:])
        sgv = sg[:, :].rearrange("p (s g) -> p g s", g=4)
        cum = ppool.tile([128, 12], f32)        # inclusive cumsums, same layout
        cumv = cum[:, :].rearrange("p (s g) -> p g s", g=4)
        tot = ppool.tile([128, 2], f32)         # total_bbox per batch, broadcast
        for b in range(NB):
            g0, g1 = 2 * b, 2 * b + 1
            ps0 = qpool.tile([128, 3], f32, tag="ps0")
            nc.tensor.matmul(out=ps0[:], lhsT=LT[:], rhs=sgv[:, g0, :], start=True, stop=True)
            nc.vector.tensor_copy(cumv[:, g0, :], ps0[:])
            ps1 = qpool.tile([128, 3], f32, tag="ps1")
            nc.tensor.matmul(out=ps1[:], lhsT=ONES[:], rhs=sgv[:, g0, :], start=True, stop=False)
            nc.tensor.matmul(out=ps1[:], lhsT=LT[:], rhs=sgv[:, g1, :], start=False, stop=True)
            nc.vector.tensor_copy(cumv[:, g1, :], ps1[:])
            pst = qpool.tile([128, 1], f32, tag="pst")
            nc.tensor.matmul(out=pst[:], lhsT=ONES[:], rhs=sg[:, 4 + g0 : 5 + g0], start=True, stop=False)
            nc.tensor.matmul(out=pst[:], lhsT=ONES[:], rhs=sg[:, 4 + g1 : 5 + g1], start=False, stop=True)
            nc.vector.tensor_copy(tot[:, b : b + 1], pst[:])
        # m4[b] = 1 if total_bbox % 4 == 0 (total appears in {0,4,...,396})
        m4 = ppool.tile([128, 2], f32)
        for b in range(NB):
            eqm = wpool.tile([128, NQ], f32, tag="eqm")
            nc.vector.tensor_scalar(eqm[:], iota4[:], tot[:, b : b + 1], None, OP.is_equal)
            nc.vector.tensor_reduce(out=m4[:, b : b + 1], in_=eqm[:], axis=AX.X, op=OP.max)

        for b in range(NB):
            hpsum = hpool.tile([NQ, NP], f32, tag="hp")
            for h in range(2):
                g = 2 * b + h
                # bbox_cnt_excl and text_pos (exclusive cumsums)
                cntg = wpool.tile([128, 1], f32, tag=f"cnt{h}")
                nc.vector.tensor_sub(cntg[:], cumv[:, g, 1:2], sg[:, 4 + g : 5 + g])
                tposg = wpool.tile([128, 1], f32, tag=f"tp{h}")
                nc.vector.tensor_sub(tposg[:], cumv[:, g, 2:3], sg[:, 8 + g : 9 + g])
                # valid = is_text & (in_bbox != 0) & !(cnt == tot & tot % 4 == 0)
                nzg = wpool.tile([128, 1], f32, tag=f"nz{h}")
                nc.vector.tensor_scalar(nzg[:], cumv[:, g, 0:1], 0.0, None, OP.not_equal)
                eqct = wpool.tile([128, 1], f32, tag=f"eqct{h}")
                nc.vector.tensor_scalar(eqct[:], cntg[:], tot[:, b : b + 1], None, OP.is_equal)
                ofg = wpool.tile([128, 1], f32, tag=f"of{h}")
                nc.vector.tensor_mul(ofg[:], eqct[:], m4[:, b : b + 1])
                nofg = wpool.tile([128, 1], f32, tag=f"nof{h}")
                nc.vector.tensor_scalar(nofg[:], ofg[:], -1.0, 1.0, OP.mult, OP.add)
                vg = wpool.tile([128, 1], f32, tag=f"v{h}")
                nc.vector.tensor_mul(vg[:], sg[:, 8 + g : 9 + g], nzg[:])
                vg2 = wpool.tile([128, 1], f32, tag=f"v2{h}")
                nc.vector.tensor_mul(vg2[:], vg[:], nofg[:])
                # R[t, q] = valid & (box_idx == q), box_idx = cnt // 4 without floor:
                #   (4q <= cnt) & (4q >= cnt - 3)
                Ag = wpool.tile([128, NQ], f32, tag=f"A{h}")
                nc.vector.tensor_scalar(Ag[:], iota4[:], cntg[:, 0:1], None, OP.is_le)
                Av = wpool.tile([128, NQ], f32, tag=f"Av{h}")
                nc.vector.tensor_scalar(Av[:], Ag[:], vg2[:, 0:1], None, OP.mult)
                cm3 = wpool.tile([128, 1], f32, tag=f"cm3{h}")
                nc.vector.tensor_scalar(cm3[:], cntg[:], 3.0, None, OP.subtract)
                Bg = wpool.tile([128, NQ], f32, tag=f"B{h}")
                nc.vector.tensor_scalar(Bg[:], iota4[:], cm3[:, 0:1], None, OP.is_ge)
                Rg = wpool.tile([128, NQ], f32, tag=f"R{h}")
                nc.vector.tensor_mul(Rg[:], Av[:], Bg[:])
                # C[t, c] = (c == text_pos + 1)
                colv = wpool.tile([128, 1], f32, tag=f"col{h}")
                nc.vector.tensor_scalar(colv[:], tposg[:], 1.0, None, OP.add)
                Cg = wpool.tile([128, NP], f32, tag=f"C{h}")
                nc.vector.tensor_scalar(Cg[:], iota256[:], colv[:, 0:1], None, OP.is_equal)
                nc.tensor.matmul(out=hpsum[:], lhsT=Rg[:, 0:NQ], rhs=Cg[:], start=(h == 0), stop=(h == 1))
            # scores = 1 - exp(onehot[:, 255]) / sum(exp(onehot))
            expo = wpool.tile([NQ, NP], f32, tag="expo")
            se = wpool.tile([NQ, 1], f32, tag="se")
            nc.scalar.activation(out=expo[:], in_=hpsum[:], func=ACTF.Exp, accum_out=se[:])
            rs = wpool.tile([NQ, 1], f32, tag="rs")
            nc.vector.reciprocal(rs[:], se[:])
            st = wpool.tile([NQ, 1], f32, tag="st")
            nc.vector.tensor_mul(st[:], expo[:, NP - 1 : NP], rs[:])
            st2 = wpool.tile([NQ, 1], f32, tag="st2")
            nc.vector.tensor_scalar(st2[:], st[:], -1.0, 1.0, OP.mult, OP.add)
            nc.sync.dma_start(out=scores.ap()[b].unsqueeze(1), in_=st2[:])

        # ---------------- phase 4: boxes ----------------
        bxt = ppool.tile([NQ, 8], f32)          # [q, (b, cxcywh)]
        nc.sync.dma_start(
            out=bxt[:, :].rearrange("q (b f) -> q b f", b=NB),
            in_=bx.ap().transpose([1, 0, 2]),
        )
        s1 = ppool.tile([1, 8], f32)
        nc.sync.dma_start(
            out=s1[:, :],
            in_=sc.ap().rearrange("b f -> (b f)").unsqueeze(0),
        )
        scb = ppool.tile([NQ, 8], f32)
        ps_sc = qpool.tile([NQ, 8], f32, tag="pssc")
        nc.tensor.matmul(out=ps_sc[:], lhsT=ONES[0:1, 0:NQ], rhs=s1[0:1, :], start=True, stop=True)
        nc.vector.tensor_copy(scb[:], ps_sc[:])
        xyo = ppool.tile([NQ, 8], f32)
        for b in range(NB):
            cxy = bxt[:, b * 4 : b * 4 + 2]
            wh = bxt[:, b * 4 + 2 : b * 4 + 4]
            nc.vector.scalar_tensor_tensor(
                out=xyo[:, b * 4 : b * 4 + 2], in0=wh, scalar=-0.5, in1=cxy,
                op0=OP.mult, op1=OP.add,
            )
            nc.vector.scalar_tensor_tensor(
                out=xyo[:, b * 4 + 2 : b * 4 + 4], in0=wh, scalar=0.5, in1=cxy,
                op0=OP.mult, op1=OP.add,
            )
        obx = ppool.tile([NQ, 8], f32)
        nc.vector.tensor_mul(obx[:], xyo[:], scb[:])
        nc.sync.dma_start(
            out=oboxes.ap().transpose([1, 0, 2]),
            in_=obx[:, :].rearrange("q (b f) -> q b f", b=NB),
        )

    nc.compile()
    return nc


def get_nc():
    if "nc" not in _CACHE:
        _CACHE["nc"] = _build()
    return _CACHE["nc"]


def make_in_maps(pred_logits, pred_boxes, target_sizes):
    pred_logits = np.ascontiguousarray(pred_logits, dtype=np.float32)
    pred_boxes = np.ascontiguousarray(pred_boxes, dtype=np.float32)
    target_sizes = np.ascontiguousarray(target_sizes, dtype=np.float32)
    B = pred_logits.shape[0]
    assert B == NB * NCORES
    img_h = target_sizes[:, 0]
    img_w = target_sizes[:, 1]
    scale = np.stack([img_w, img_h, img_w, img_h], axis=1)   # [B, 4]
    pad = np.zeros(PADF, np.float32)
    in_maps = []
    for i in range(NCORES):
        b0 = NB * i
        in_maps.append(
            {
                "lg": np.concatenate([pred_logits[b0 : b0 + NB].reshape(-1), pad]),
                "bx": pred_boxes[b0 : b0 + NB],
                "sc": scale[b0 : b0 + NB],
            }
        )
    return in_maps


def kernel(pred_logits, pred_boxes, target_sizes):
    nc = get_nc()
    in_maps = make_in_maps(pred_logits, pred_boxes, target_sizes)
    res = run_bass_kernel_spmd(nc, in_maps, core_ids=list(range(NCORES))).results
    B = pred_logits.shape[0]
    scores = np.concatenate([res[i]["scores"] for i in range(NCORES)], axis=0)
    boxes = np.concatenate([res[i]["oboxes"] for i in range(NCORES)], axis=0)
    labels = np.ones((B, NQ), dtype=np.int32)
    return scores, labels, boxes


# revision 10
# speedup vs baseline: 193.3191x; 193.3191x over previous
"""Trainium2 Bass kernel for the PostProcess problem.

Computation (per batch element):
  pred = argmax(pred_logits[b], axis=-1)          # [L] over V=51267
  token state machine -> one-hot return_logits [100, 256]
  scores = 1 - softmax(return_logits)[..., -1]
  boxes  = cxcywh_to_xyxy(pred_boxes) * [w,h,w,h]
  labels = ones

Sharding: pure data parallel, 2 batch elements per core on 8 cores.

Argmax strategy (memory-bound regime): stream the logits through SBUF once,
reduce to per-128-element-subchunk maxima on DVE (1 full scan), find the
winning subchunk with tiny max/max_index ops, then re-fetch just the winning
512B subchunk per row via indirect DMA and max_index it.  DVE work ~1.03
scans instead of 2 (max+max_index over everything), so the kernel stays
DMA-bound at ~105MB/core.
"""

import sys

if "/opt/trn_rl_repo" not in sys.path:
    sys.path.insert(0, "/opt/trn_rl_repo")

from contextlib import ExitStack

import numpy as np

import concourse.bacc as bacc
import concourse.bass as bass
import concourse.tile as tile
from concourse import mybir
from concourse.bass_utils import run_bass_kernel_spmd

V = 51267
L = 256
NB = 2              # batch elements per core
RW = NB * L         # 512 token rows per core
NCORES = 8
TEXT = 50265.0      # text vocab size; open=50265, close=50266, bbox>50266
NQ = 100
NP = 256
SUB = 128           # subchunk size for hierarchical argmax
SFULL = 400         # full subchunks per row (400*128 = 51200)
TAIL = V - SFULL * SUB   # 67
STOT = SFULL + 1    # 401 subchunk maxima per row
NFLAT = RW * V
CH_OFF = [0, 12800, 25600, 38400]
CH_COLS = [12800, 12800, 12800, 12867]

f32 = mybir.dt.float32
i32 = mybir.dt.int32
u32 = mybir.dt.uint32
OP = mybir.AluOpType
AX = mybir.AxisListType
ACTF = mybir.ActivationFunctionType

_CACHE = {}


def _build():
    nc = bacc.Bacc("TRN2", target_bir_lowering=False, debug=False)
    lg = nc.dram_tensor("lg", [NFLAT], f32, kind="ExternalInput")
    bx = nc.dram_tensor("bx", [NB, NQ, 4], f32, kind="ExternalInput")
    sc = nc.dram_tensor("sc", [NB, 4], f32, kind="ExternalInput")
    scores = nc.dram_tensor("scores", [NB, NQ], f32, kind="ExternalOutput")
    oboxes = nc.dram_tensor("oboxes", [NB, NQ, 4], f32, kind="ExternalOutput")

    lgap = lg.ap()                                      # [NFLAT]
    lgm = lgap[0 : RW * V].rearrange("(r v) -> r v", v=V)   # [512, V]
    lgcol = lgap.unsqueeze(1)                           # [NFLAT, 1]

    with tile.TileContext(nc) as tc, ExitStack() as ctx:
        cpool = ctx.enter_context(tc.tile_pool(name="consts", bufs=1))
        spool = ctx.enter_context(tc.tile_pool(name="stream", bufs=3))
        mpool = ctx.enter_context(tc.tile_pool(name="smax", bufs=2))
        wpool = ctx.enter_context(tc.tile_pool(name="work", bufs=2))
        ppool = ctx.enter_context(tc.tile_pool(name="persist", bufs=1))
        qpool = ctx.enter_context(tc.tile_pool(name="psmall", bufs=1, space="PSUM"))
        hpool = ctx.enter_context(tc.tile_pool(name="phot", bufs=2, space="PSUM"))

        # ---------------- constants ----------------
        it_a = cpool.tile([128, 1], i32)
        it_b = cpool.tile([128, NQ], i32)
        it_c = cpool.tile([128, NP], i32)
        it_d = cpool.tile([128, 128], i32)
        it_e = cpool.tile([128, 1], i32)
        iota_pv = cpool.tile([128, 1], f32)     # p * V
        nc.gpsimd.iota(it_a[:], [[1, 1]], channel_multiplier=V)
        nc.vector.tensor_copy(iota_pv[:], it_a[:])
        iota4 = cpool.tile([128, NQ], f32)      # 0,4,...,396 per partition
        nc.gpsimd.iota(it_b[:], [[4, NQ]], channel_multiplier=0)
        nc.vector.tensor_copy(iota4[:], it_b[:])
        iota256 = cpool.tile([128, NP], f32)    # 0..255 per partition
        nc.gpsimd.iota(it_c[:], [[1, NP]], channel_multiplier=0)
        nc.vector.tensor_copy(iota256[:], it_c[:])
        irow = cpool.tile([128, 128], f32)      # 0..127 per partition
        nc.gpsimd.iota(it_d[:], [[1, 128]], channel_multiplier=0)
        nc.vector.tensor_copy(irow[:], it_d[:])
        pcol = cpool.tile([128, 1], f32)        # partition index
        nc.gpsimd.iota(it_e[:], [[1, 1]], channel_multiplier=1)
        nc.vector.tensor_copy(pcol[:], it_e[:])
        # LT[k, m] = 1 if k <= m  (inclusive prefix-sum matrix)
        LT = cpool.tile([128, 128], f32)
        nc.vector.tensor_scalar(LT[:], irow[:], pcol[:, 0:1], None, OP.is_ge)
        ONES = cpool.tile([128, 128], f32)
        nc.vector.memset(ONES[:], 1.0)

        # ---------------- phase 1+2: streaming subchunk max + argmax ----------------
        pred_all = ppool.tile([128, 4], f32)    # col r: token rows r*128..r*128+127
        for r in range(4):
            smax = mpool.tile([128, STOT], f32, tag="smax")
            for c in range(4):
                cols = CH_COLS[c]
                X = spool.tile([128, 12867], f32, tag="X")
                nc.sync.dma_start(
                    out=X[:, 0:cols],
                    in_=lgm[r * 128 : (r + 1) * 128, CH_OFF[c] : CH_OFF[c] + cols],
                )
                nc.vector.tensor_reduce(
                    out=smax[:, c * 100 : (c + 1) * 100],
                    in_=X[:, 0:12800].rearrange("p (s k) -> p s k", k=SUB),
                    axis=AX.X,
                    op=OP.max,
                )
                if c == 3:
                    nc.vector.tensor_reduce(
                        out=smax[:, SFULL : SFULL + 1],
                        in_=X[:, 12800:12867],
                        axis=AX.X,
                        op=OP.max,
                    )
            top8 = wpool.tile([128, 8], f32, tag="top8")
            kidx8 = wpool.tile([128, 8], u32, tag="kidx8")
            nc.vector.max(out=top8[:], in_=smax[:, 0:STOT])
            nc.vector.max_index(out=kidx8[:], in_max=top8[:], in_values=smax[:, 0:STOT])
            kf = wpool.tile([128, 1], f32, tag="kf")
            nc.vector.tensor_copy(kf[:], kidx8[:, 0:1].bitcast(i32))
            # window start = min(128*k, V-128): the tail window is pulled back so
            # the gather stays inside the row (max_index still finds the true
            # max first since earlier-subchunk values are strictly smaller)
            st0 = wpool.tile([128, 1], f32, tag="st0")
            nc.vector.tensor_scalar(st0[:], kf[:], float(SUB), float(V - SUB), OP.mult, OP.min)
            offf = wpool.tile([128, 1], f32, tag="offf")
            nc.vector.tensor_add(offf[:], st0[:], iota_pv[:])
            offi = wpool.tile([128, 1], i32, tag="offi")
            nc.vector.tensor_copy(offi[:], offf[:])
            gath = wpool.tile([128, SUB], f32, tag="gath")
            nc.gpsimd.indirect_dma_start(
                out=gath[:],
                out_offset=None,
                in_=lgcol,
                in_offset=bass.IndirectOffsetOnAxis(ap=offi[:, 0:1], axis=0),
                element_offset=r * 128 * V,
            )
            widx8 = wpool.tile([128, 8], u32, tag="widx8")
            nc.vector.max_index(out=widx8[:], in_max=top8[:], in_values=gath[:])
            wf = wpool.tile([128, 1], f32, tag="wf")
            nc.vector.tensor_copy(wf[:], widx8[:, 0:1].bitcast(i32))
            nc.vector.tensor_add(pred_all[:, r : r + 1], st0[:], wf[:])

        # ---------------- phase 3: token state machine ----------------
        # group g = r: batch b = g//2, half h = g%2 (token l = h*128 + p)
        iso = ppool.tile([128, 4], f32)
        isc = ppool.tile([128, 4], f32)
        isb = ppool.tile([128, 4], f32)
        ist = ppool.tile([128, 4], f32)
        nc.vector.tensor_scalar(iso[:], pred_all[:], TEXT, None, OP.is_equal)
        nc.vector.tensor_scalar(isc[:], pred_all[:], TEXT + 1.0, None, OP.is_equal)
        nc.vector.tensor_scalar(isb[:], pred_all[:], TEXT + 1.0, None, OP.is_gt)
        nc.vector.tensor_scalar(ist[:], pred_all[:], TEXT, None, OP.is_lt)
        # signals tile: cols [0:4]=delta(g), [4:8]=is_bbox(g), [8:12]=is_text(g)
        sg = ppool.tile([128, 12], f32)
        nc.vector.tensor_sub(sg[:, 0:4], iso[:], isc[:])
        nc.vector.tensor_copy(sg[:, 4:8], isb[:])
        nc.vector.tensor_copy(sg[:, 8:12], ist[:])
        sgv = sg[:, :].rearrange("p (s g) -> p g s", g=4)
        cum = ppool.tile([128, 12], f32)        # inclusive cumsums, same layout
        cumv = cum[:, :].rearrange("p (s g) -> p g s", g=4)
        tot = ppool.tile([128, 2], f32)         # total_bbox per batch, broadcast
        for b in range(NB):
            g0, g1 = 2 * b, 2 * b + 1
            ps0 = qpool.tile([128, 3], f32, tag="ps0")
            nc.tensor.matmul(out=ps0[:], lhsT=LT[:], rhs=sgv[:, g0, :], start=True, stop=True)
            nc.vector.tensor_copy(cumv[:, g0, :], ps0[:])
            ps1 = qpool.tile([128, 3], f32, tag="ps1")
            nc.tensor.matmul(out=ps1[:], lhsT=ONES[:], rhs=sgv[:, g0, :], start=True, stop=False)
            nc.tensor.matmul(out=ps1[:], lhsT=LT[:], rhs=sgv[:, g1, :], start=False, stop=True)
            nc.vector.tensor_copy(cumv[:, g1, :], ps1[:])
            pst = qpool.tile([128, 1], f32, tag="pst")
            nc.tensor.matmul(out=pst[:], lhsT=ONES[:], rhs=sg[:, 4 + g0 : 5 + g0], start=True, stop=False)
            nc.tensor.matmul(out=pst[:], lhsT=ONES[:], rhs=sg[:, 4 + g1 : 5 + g1], start=False, stop=True)
            nc.vector.tensor_copy(tot[:, b : b + 1], pst[:])
        # m4[b] = 1 if total_bbox % 4 == 0 (total appears in {0,4,...,396})
        m4 = ppool.tile([128, 2], f32)
        for b in range(NB):
            eqm = wpool.tile([128, NQ], f32, tag="eqm")
            nc.vector.tensor_scalar(eqm[:], iota4[:], tot[:, b : b + 1], None, OP.is_equal)
            nc.vector.tensor_reduce(out=m4[:, b : b + 1], in_=eqm[:], axis=AX.X, op=OP.max)

        for b in range(NB):
            hpsum = hpool.tile([NQ, NP], f32, tag="hp")
            for h in range(2):
                g = 2 * b + h
                # bbox_cnt_excl and text_pos (exclusive cumsums)
                cntg = wpool.tile([128, 1], f32, tag=f"cnt{h}")
                nc.vector.tensor_sub(cntg[:], cumv[:, g, 1:2], sg[:, 4 + g : 5 + g])
                tposg = wpool.tile([128, 1], f32, tag=f"tp{h}")
                nc.vector.tensor_sub(tposg[:], cumv[:, g, 2:3], sg[:, 8 + g : 9 + g])
                # valid = is_text & (in_bbox != 0) & !(cnt == tot & tot % 4 == 0)
                nzg = wpool.tile([128, 1], f32, tag=f"nz{h}")
                nc.vector.tensor_scalar(nzg[:], cumv[:, g, 0:1], 0.0, None, OP.not_equal)
                eqct = wpool.tile([128, 1], f32, tag=f"eqct{h}")
                nc.vector.tensor_scalar(eqct[:], cntg[:], tot[:, b : b + 1], None, OP.is_equal)
                ofg = wpool.tile([128, 1], f32, tag=f"of{h}")
                nc.vector.tensor_mul(ofg[:], eqct[:], m4[:, b : b + 1])
                nofg = wpool.tile([128, 1], f32, tag=f"nof{h}")
                nc.vector.tensor_scalar(nofg[:], ofg[:], -1.0, 1.0, OP.mult, OP.add)
                vg = wpool.tile([128, 1], f32, tag=f"v{h}")
                nc.vector.tensor_mul(vg[:], sg[:, 8 + g : 9 + g], nzg[:])
                vg2 = wpool.tile([128, 1], f32, tag=f"v2{h}")
                nc.vector.tensor_mul(vg2[:], vg[:], nofg[:])
                # R[t, q] = valid & (box_idx == q), box_idx = cnt // 4 without floor:
                #   (4q <= cnt) & (4q >= cnt - 3)
                Ag = wpool.tile([128, NQ], f32, tag=f"A{h}")
                nc.vector.tensor_scalar(Ag[:], iota4[:], cntg[:, 0:1], None, OP.is_le)
                Av = wpool.tile([128, NQ], f32, tag=f"Av{h}")
                nc.vector.tensor_scalar(Av[:], Ag[:], vg2[:, 0:1], None, OP.mult)
                cm3 = wpool.tile([128, 1], f32, tag=f"cm3{h}")
                nc.vector.tensor_scalar(cm3[:], cntg[:], 3.0, None, OP.subtract)
                Bg = wpool.tile([128, NQ], f32, tag=f"B{h}")
                nc.vector.tensor_scalar(Bg[:], iota4[:], cm3[:, 0:1], None, OP.is_ge)
                Rg = wpool.tile([128, NQ], f32, tag=f"R{h}")
                nc.vector.tensor_mul(Rg[:], Av[:], Bg[:])
                # C[t, c] = (c == text_pos + 1)
                colv = wpool.tile([128, 1], f32, tag=f"col{h}")
                nc.vector.tensor_scalar(colv[:], tposg[:], 1.0, None, OP.add)
                Cg = wpool.tile([128, NP], f32, tag=f"C{h}")
                nc.vector.tensor_scalar(Cg[:], iota256[:], colv[:, 0:1], None, OP.is_equal)
                nc.tensor.matmul(out=hpsum[:], lhsT=Rg[:, 0:NQ], rhs=Cg[:], start=(h == 0), stop=(h == 1))
            # scores = 1 - exp(onehot[:, 255]) / sum(exp(onehot))
            expo = wpool.tile([NQ, NP], f32, tag="expo")
            se = wpool.tile([NQ, 1], f32, tag="se")
            nc.scalar.activation(out=expo[:], in_=hpsum[:], func=ACTF.Exp, accum_out=se[:])
            rs = wpool.tile([NQ, 1], f32, tag="rs")
            nc.vector.reciprocal(rs[:], se[:])
            st = wpool.tile([NQ, 1], f32, tag="st")
            nc.vector.tensor_mul(st[:], expo[:, NP - 1 : NP], rs[:])
            st2 = wpool.tile([NQ, 1], f32, tag="st2")
            nc.vector.tensor_scalar(st2[:], st[:], -1.0, 1.0, OP.mult, OP.add)
            nc.sync.dma_start(out=scores.ap()[b].unsqueeze(1), in_=st2[:])

        # ---------------- phase 4: boxes ----------------
        bxt = ppool.tile([NQ, 8], f32)          # [q, (b, cxcywh)]
        nc.sync.dma_start(
            out=bxt[:, :].rearrange("q (b f) -> q b f", b=NB),
            in_=bx.ap().transpose([1, 0, 2]),
        )
        s1 = ppool.tile([1, 8], f32)
        nc.sync.dma_start(
            out=s1[:, :],
            in_=sc.ap().rearrange("b f -> (b f)").unsqueeze(0),
        )
        scb = ppool.tile([NQ, 8], f32)
        ps_sc = qpool.tile([NQ, 8], f32, tag="pssc")
        nc.tensor.matmul(out=ps_sc[:], lhsT=ONES[0:1, 0:NQ], rhs=s1[0:1, :], start=True, stop=True)
        nc.vector.tensor_copy(scb[:], ps_sc[:])
        xyo = ppool.tile([NQ, 8], f32)
        for b in range(NB):
            cxy = bxt[:, b * 4 : b * 4 + 2]
            wh = bxt[:, b * 4 + 2 : b * 4 + 4]
            nc.vector.scalar_tensor_tensor(
                out=xyo[:, b * 4 : b * 4 + 2], in0=wh, scalar=-0.5, in1=cxy,
                op0=OP.mult, op1=OP.add,
            )
            nc.vector.scalar_tensor_tensor(
                out=xyo[:, b * 4 + 2 : b * 4 + 4], in0=wh, scalar=0.5, in1=cxy,
                op0=OP.mult, op1=OP.add,
            )
        obx = ppool.tile([NQ, 8], f32)
        nc.vector.tensor_mul(obx[:], xyo[:], scb[:])
        nc.sync.dma_start(
            out=oboxes.ap().transpose([1, 0, 2]),
            in_=obx[:, :].rearrange("q (b f) -> q b f", b=NB),
        )

    nc.compile()
    return nc


def get_nc():
    if "nc" not in _CACHE:
        _CACHE["nc"] = _build()
    return _CACHE["nc"]


def get_runner():
    """One jitted shard_map executable, cached across kernel() calls."""
    if "runner" in _CACHE:
        return _CACHE["runner"]
    import jax
    from jax.experimental.shard_map import shard_map
    from jax.sharding import Mesh, PartitionSpec

    from concourse import bass2jax

    nc = get_nc()
    bass2jax.install_neuronx_cc_hook()
    partition_name = nc.partition_id_tensor.name if nc.partition_id_tensor else None
    in_names, out_names, out_avals = [], [], []
    for alloc in nc.m.functions[0].allocations:
        if not isinstance(alloc, mybir.MemoryLocationSet):
            continue
        name = alloc.memorylocations[0].name
        if alloc.kind == "ExternalInput":
            if name != partition_name:
                in_names.append(name)
        elif alloc.kind == "ExternalOutput":
            out_names.append(name)
            out_avals.append(
                jax.core.ShapedArray(
                    tuple(alloc.tensor_shape), mybir.dt.np(alloc.dtype)
                )
            )
    n_params = len(in_names)
    n_outs = len(out_names)
    all_in_names = list(in_names) + list(out_names)
    if partition_name is not None:
        all_in_names.append(partition_name)
    donate = tuple(range(n_params, n_params + n_outs))

    def _body(*args):
        operands = list(args)
        if partition_name is not None:
            operands.append(bass2jax.partition_id_tensor())
        outs = bass2jax._bass_exec_p.bind(
            *operands,
            out_avals=tuple(out_avals),
            in_names=tuple(all_in_names),
            out_names=tuple(out_names),
            lowering_input_output_aliases=(),
            sim_require_finite=True,
            sim_require_nnan=True,
            nc=nc,
        )
        return tuple(outs)

    devices = jax.devices()[:NCORES]
    mesh = Mesh(np.asarray(devices), ("core",))
    in_specs = (PartitionSpec("core"),) * (n_params + n_outs)
    out_specs = (PartitionSpec("core"),) * n_outs
    sharded = jax.jit(
        shard_map(
            _body, mesh=mesh, in_specs=in_specs, out_specs=out_specs, check_rep=False
        ),
        donate_argnums=donate,
        keep_unused=True,
    )
    _CACHE["runner"] = (sharded, in_names, out_names, out_avals)
    return _CACHE["runner"]


def make_global_inputs(pred_logits, pred_boxes, target_sizes):
    """Global (concatenated-over-cores) input arrays, keyed by dram name."""
    pred_logits = np.ascontiguousarray(pred_logits, dtype=np.float32)
    pred_boxes = np.ascontiguousarray(pred_boxes, dtype=np.float32)
    target_sizes = np.ascontiguousarray(target_sizes, dtype=np.float32)
    B = pred_logits.shape[0]
    assert B == NB * NCORES
    img_h = target_sizes[:, 0]
    img_w = target_sizes[:, 1]
    scale = np.stack([img_w, img_h, img_w, img_h], axis=1)   # [B, 4]
    # zero-copy: global "lg" is just the flat logits; shard_map hands each
    # core a [NB*L*V] slice
    return {
        "lg": pred_logits.reshape(NCORES * NFLAT),
        "bx": pred_boxes,
        "sc": scale,
    }


def run_global(gin):
    sharded, in_names, out_names, out_avals = get_runner()
    concat_in = [gin[n] for n in in_names]
    concat_zeros = [
        np.zeros((NCORES * a.shape[0], *a.shape[1:]), a.dtype) for a in out_avals
    ]
    out_arrs = sharded(*concat_in, *concat_zeros)
    return dict(zip(out_names, out_arrs))


def kernel(pred_logits, pred_boxes, target_sizes):
    B = pred_logits.shape[0]
    gin = make_global_inputs(pred_logits, pred_boxes, target_sizes)
    outs = run_global(gin)
    scores = np.asarray(outs["scores"])          # [B, NQ]
    boxes = np.asarray(outs["oboxes"])           # [B, NQ, 4]
    labels = np.ones((B, NQ), dtype=np.int32)
    return scores, labels, boxes
